# revision 22
# baseline (speedup 1.0000x reference)
"""Trainium2 Bass kernel for nn_DeepQNetwork (dense_mlp).

Reference computation (per row of x [B, 15]):
    keep = x[:, :11]
    hold_oh = one_hot(int(x[:, 11]), 4)
    nxt_oh  = one_hot(int(x[:, 12:15]) - 1, 7) each  -> 21 cols
    inp = [keep, hold_oh, nxt_oh]            # [B, 36]
    h1 = relu(inp @ W1 + b1)                 # [B, 128]
    h2 = relu(h1 @ W2 + b2)                  # [B, 512]
    out = h2 @ W4 + b4                       # [B, 40]

Strategy: pure data parallel over 8 NeuronCores (batch sharded, weights
replicated).  Per-core dataflow:

  host packs x as bf16 [B, 128]: [x(15) | 0 | replicated ids(25) at 32:57 | 0]
  (128 feature columns so the XBAR DMA-transpose uses the HW-validated
  full-partition output shape [128, E, 128])
  DMA-transpose loads each supertile straight from DRAM into feature-major
  SBUF rhs [128, BT]  (no PE transpose, no PSUM traffic for the input)
  one in-place GPSIMD is_equal vs per-partition class consts turns the
  replicated id rows 32:57 into one-hot rows (32-aligned partition base —
  walrus requires it)
  L1: h1 = relu(W1p.T @ rhs + b1)            [128, BT]   (bf16 matmul)
  L2: h2_g = relu(W2_g.T @ h1 + b2_g)        [128, BT] x4
  L3 batch-major: out[128b, 40] accumulates h2_c-slice.T @ W4_c over c
     (h2 is the stationary operand; streams only N=40 cols per matmul),
     plus a K=32 ones-row matmul that adds b4 inside the PSUM accumulation
  out lands batch-major -> copy drain -> DMA store (no output transpose)

All matmuls run bf16 (1 cyc/row on the PE at any N; fp32 PSUM accumulate),
rel err ~5e-3 (fp8 measured 4.4e-2 — over the 2e-2 gate — and rejected).
PSUM drains (relu+bias+cast) are the co-bottleneck with the PE: paired into
[128, 1024] two-bank tiles, alternated ACT/DVE (GPSIMD cannot touch PSUM on
real HW), and every cross-engine dependency except the PSUM-tile rotation
trails >= half a supertile.  L3's psum uses 64-float chunk slots so no
matmul output or drain read straddles a 2 KB PSUM bank boundary.
"""

import os
import numpy as np
import ml_dtypes

try:  # persistent XLA/NEFF cache: makes fresh-process compiles fast
    import jax as _jax

    _jax.config.update("jax_compilation_cache_dir", "/tmp/jax_neff_cache")
    _jax.config.update("jax_persistent_cache_min_compile_time_secs", 1.0)
except Exception:
    pass

import concourse.bacc as bacc
import concourse.bass as bass
import concourse.mybir as mybir
import concourse.tile as tile
from concourse.bass_utils import run_bass_kernel_spmd

N_CORES = 8
B_TOTAL = 131072
B_CORE = B_TOTAL // N_CORES  # 16384
BT = 2048                    # batch per supertile
N_ST = B_CORE // BT          # supertiles per core (8)
E_CH = BT // 128             # 128-col chunks per supertile (16)
NF = 128                     # feature rows: 15 raw | pad | 25 one-hot @32 | pad

F32 = mybir.dt.float32
BF16 = mybir.dt.bfloat16

# schedule configuration (swept empirically against TimelineSim)
CONFIG = dict(
    # engine for each of the 10 pair-drain slots per supertile, in emission
    # order: halfA [L2 g0, g1, L1p0, g2, g3], halfB [L2 g0, g1, L1p1, g2, g3]
    cadence=["dve", "act", "dve", "act", "dve",
             "act", "dve", "act", "dve", "act"],
    copy_late=True,      # True: both output copies sit in half B
    pops=(3, 3, 2, 0),   # l3-chunk pops after each L2 pair-group
    h_bufs=3,            # h1/h2 tile buffering depth
    pq_bufs=3,           # [128,1024] pair-psum rotation depth
)

_BUILT = {}

# packed weight blob column layout (bf16 blob [128, 968], f32 blob [128, 6])
_WB_W1 = slice(0, 128)       # w1p [128, 128] (rows 64:128 zero)
_WB_W2 = slice(128, 640)     # w2p [128, 512]
_WB_W4 = slice(640, 800)     # w4p [128, 160]
_WB_ONE = slice(800, 928)    # ones row [1, 128] in partition 0, zeros 1:32
_WB_B4 = slice(928, 968)     # b4 row [1, 40] in partition 0, zeros 1:32
_FB_B1 = slice(0, 1)         # b1 [128, 1]
_FB_B2 = slice(1, 5)         # b2 [128, 4]
_FB_CV = slice(5, 6)         # cvec (one-hot class consts in partitions 32:64)


def _build(mode: str):
    """Build the per-core Bass module (same NEFF on all 8 cores)."""
    nc = bacc.Bacc("TRN2", target_bir_lowering=False, debug=False)
    cfg = CONFIG

    x_d = nc.dram_tensor("xrep", [B_CORE, NF], BF16, kind="ExternalInput").ap()
    wb_d = nc.dram_tensor("wblob", [128, 968], BF16, kind="ExternalInput").ap()
    fb_d = nc.dram_tensor("fblob", [128, 6], F32, kind="ExternalInput").ap()
    out_d = nc.dram_tensor("out", [B_CORE, 40], F32, kind="ExternalOutput").ap()

    from contextlib import ExitStack

    with tile.TileContext(nc) as tc, ExitStack() as ctx:
        consts = ctx.enter_context(tc.tile_pool(name="consts", bufs=1))
        wb_sb = consts.tile([128, 968], BF16, tag="wb")
        fb_sb = consts.tile([128, 6], F32, tag="fb")
        w1_sb = wb_sb[:, _WB_W1]
        w2_sb = wb_sb[:, _WB_W2]
        w4_sb = wb_sb[:, _WB_W4]
        one_sb = wb_sb[0:32, _WB_ONE]
        b4r_sb = wb_sb[0:32, _WB_B4]
        b1_sb = fb_sb[:, _FB_B1]
        b2_sb = fb_sb[:, _FB_B2]
        cv_sb = fb_sb[:, _FB_CV]

        rhs_pool = ctx.enter_context(tc.tile_pool(name="rhs", bufs=3))
        h1_pool = ctx.enter_context(tc.tile_pool(name="h1", bufs=cfg["h_bufs"]))
        h2_pool = ctx.enter_context(tc.tile_pool(name="h2", bufs=cfg["h_bufs"]))
        osb_pool = ctx.enter_context(tc.tile_pool(name="osb", bufs=4))
        # PSUM: 8 banks = pair-drain pool 3x[128,1024] (6) + p3 (2)
        pqf_pool = ctx.enter_context(
            tc.tile_pool(name="pqf", bufs=cfg["pq_bufs"], space="PSUM"))
        p3_pool = ctx.enter_context(tc.tile_pool(name="p3", bufs=1, space="PSUM"))
        # 64-float slots: 8 chunks fill each 2KB psum bank exactly, so no
        # matmul output or drain read straddles a bank boundary (HW trap)
        p3 = p3_pool.tile([128, E_CH, 64], F32, tag="p3")

        # weight loads are issued around dma_t(0)/dma_t(1) in the prologue,
        # split across sequencers so HWDGE serialization doesn't delay the
        # first supertile: w1 (needed first) rides its own small DMA.
        def load_weights():
            nc.scalar.dma_start(fb_sb[:], fb_d)
            nc.sync.dma_start(wb_sb[:, _WB_W1], wb_d[:, _WB_W1])
            nc.scalar.dma_start(wb_sb[:, 128:], wb_d[:, 128:])

        S = {}  # per-supertile state

        cadence = list(cfg["cadence"])
        drain_i = [0]

        def next_drain_eng():
            eng = cadence[drain_i[0] % len(cadence)]
            drain_i[0] += 1
            return eng

        def relu_bias(eng, out_ap, psum_ap, bias_ap):
            if eng == "act":
                nc.scalar.activation(
                    out_ap, psum_ap, mybir.ActivationFunctionType.Relu,
                    bias=bias_ap, scale=1.0,
                )
            else:
                nc.vector.tensor_scalar(
                    out_ap, psum_ap, bias_ap, 0.0,
                    op0=mybir.AluOpType.add, op1=mybir.AluOpType.max,
                )

        def dma_t(st):
            if not (0 <= st < N_ST):
                return
            xv = x_d[st * BT:(st + 1) * BT, :].rearrange(
                "(p e) f -> p (e f)", p=128
            )
            rhs = rhs_pool.tile([NF, E_CH, 128], BF16, tag="rhs", name=f"rhs{st}")
            nc.sync.dma_start_transpose(rhs[:], xv)
            S[st] = {"rhs": rhs}

        def iseq(st):
            # one-hot in place on GPSIMD (SBUF-only engine; keeps ACT/DVE
            # free for PSUM drains): rows 32:64 become (v == class_const);
            # pad rows compare vs -1 so they are always written 0.
            if not (0 <= st < N_ST):
                return
            nc.gpsimd.tensor_scalar(
                S[st]["rhs"][32:64], S[st]["rhs"][32:64], cv_sb[32:64], None,
                op0=mybir.AluOpType.is_equal,
            )

        def l1pair(st, half):
            # L1 slices (2*half, 2*half+1): one 2-bank psum tile, one drain
            if not (0 <= st < N_ST):
                return
            d = S[st]
            if "h1" not in d:
                d["h1"] = h1_pool.tile(
                    [128, E_CH, 128], BF16, tag="h1", name=f"h1_{st}"
                )
            pq = pqf_pool.tile([128, 1024], F32, tag="pq", name=f"p1_{st}_{half}")
            for i in range(2):
                s = 2 * half + i
                nc.tensor.matmul(
                    pq[:, i * 512:(i + 1) * 512], w1_sb,
                    d["rhs"][:, 4 * s:4 * s + 4, :], start=True, stop=True,
                )
            h1v = d["h1"][:, 8 * half:8 * half + 8, :].rearrange(
                "p e f -> p (e f)"
            )
            relu_bias(next_drain_eng(), h1v, pq[:], b1_sb)

        def l2pair(st, half, g):
            # L2 group g, slices (2*half, 2*half+1): 2 matmuls, one drain
            if not (0 <= st < N_ST):
                return
            d = S[st]
            h2 = d.setdefault("h2", {})
            if g not in h2:
                h2[g] = h2_pool.tile(
                    [128, E_CH, 128], BF16, tag=f"h2_{g}", name=f"h2_{st}_{g}"
                )
            pq = pqf_pool.tile([128, 1024], F32, tag="pq",
                               name=f"p2_{st}_{half}_{g}")
            for i in range(2):
                s = 2 * half + i
                nc.tensor.matmul(
                    pq[:, i * 512:(i + 1) * 512],
                    w2_sb[:, g * 128:(g + 1) * 128],
                    d["h1"][:, 4 * s:4 * s + 4, :], start=True, stop=True,
                )
            h2v = h2[g][:, 8 * half:8 * half + 8, :].rearrange(
                "p e f -> p (e f)"
            )
            relu_bias(next_drain_eng(), h2v, pq[:], b2_sb[:, g:g + 1])

        def l3_chunk(st, e):
            # L3 batch-major col-chunk e: p3[:, e, 0:40] = b4 +
            # sum_c h2_c[:, e].T @ W4_c  (h2 stationary, W4/b4 stream N=40)
            if not (0 <= st < N_ST):
                return
            d = S[st]
            for c in range(4):
                nc.tensor.matmul(
                    p3[:, e, 0:40], d["h2"][c][:, e, :],
                    w4_sb[:, c * 40:(c + 1) * 40],
                    start=(c == 0), stop=False,
                )
            nc.tensor.matmul(
                p3[:, e, 0:40], one_sb, b4r_sb, start=False, stop=True,
            )

        def out_half(st, half):
            # copy drain + store: o[128, 8, 40] -> DRAM rows (p*16 + e)
            if not (0 <= st < N_ST):
                return
            o_sb = osb_pool.tile([128, 8, 40], F32, tag="osb",
                                 name=f"o_{st}_{half}")
            p3v = p3[:, 8 * half:8 * half + 8, 0:40]
            if half == 0:
                nc.scalar.copy(o_sb[:], p3v)
            else:
                nc.vector.tensor_copy(o_sb[:], p3v)
            ov = out_d[st * BT:(st + 1) * BT, :].rearrange(
                "(p e) f -> p e f", p=128
            )[:, half * 8:half * 8 + 8, :]
            nc.sync.dma_start(ov, o_sb[:])
            if half == 1 and st - 1 in S:
                del S[st - 1]

        # --- software pipeline ----------------------------------------
        # dma_t runs 2 supertiles ahead; one-hot 2 ahead; L1 of st+1 runs
        # inside st; L3 chunks of st-1 run inside st, interleaved between
        # L2 pairs so every psum tile's reuse distance exceeds its drain's
        # end-to-end latency.
        chunkq = []

        def pop_chunk(n):
            for _ in range(min(n, len(chunkq))):
                l3_chunk(*chunkq.pop(0))

        pops = list(cfg["pops"])
        copy_late = cfg["copy_late"]

        dma_t(0)
        load_weights()
        dma_t(1)
        iseq(0)
        l1pair(0, 0)
        iseq(1)
        l1pair(0, 1)
        drain_i[0] = 0  # prologue L1 drains don't shift the cadence
        for st in range(N_ST):
            dma_t(st + 2)
            if st >= 1:
                chunkq.extend((st - 1, e) for e in range(0, 8))
            for g in range(4):
                l2pair(st, 0, g)
                pop_chunk(pops[g])
                if g == 1:
                    l1pair(st + 1, 0)
                elif g == 3 and not copy_late:
                    out_half(st - 1, 0)
            if st >= 1:
                chunkq.extend((st - 1, e) for e in range(8, 16))
            for g in range(4):
                l2pair(st, 1, g)
                pop_chunk(pops[g])
                if g == 0:
                    iseq(st + 2)
                    if copy_late:
                        out_half(st - 1, 0)
                elif g == 1:
                    l1pair(st + 1, 1)
                elif g == 3:
                    out_half(st - 1, 1)
        # epilogue: drain the last supertile's L3
        st = N_ST - 1
        chunkq.extend((st, e) for e in range(0, 16))
        pop_chunk(8)
        out_half(st, 0)
        pop_chunk(8)
        out_half(st, 1)

    nc.compile()
    return nc


def _prep_inputs(x, W1, b1, W2, b2, W4, b4):
    """Host-side packing: layout transforms only (replication, reorder,
    dtype casts); all arithmetic stays on-device."""
    f = np.float32
    bf = ml_dtypes.bfloat16
    x, W1, b1, W2, b2, W4, b4 = (
        np.asarray(a) for a in (x, W1, b1, W2, b2, W4, b4)
    )
    # x packed [B, 128]: raw 15 | pad | cols 32:57 replicated ids | pad
    rep_cols = [11] * 4 + [12] * 7 + [13] * 7 + [14] * 7
    xrep = np.zeros((B_TOTAL, NF), dtype=bf)
    xrep[:, 0:15] = x.astype(bf)
    xrep[:, 32:57] = x[:, rep_cols].astype(bf)
    # W1p rows: 0:11 keep | 32:36 hold one-hot | 36:57 next one-hot | else 0
    w1p = np.zeros((128, 128), f)
    w1p[0:11] = W1[0:11]
    w1p[32:36] = W1[11:15]   # hold one-hot
    w1p[36:57] = W1[15:36]   # next-piece one-hot
    # [512, 40] -> chunks c of 128 rows side by side -> [128, 4*40]
    w4p = W4.reshape(4, 128, 40).transpose(1, 0, 2).reshape(128, 160)
    wblob = np.zeros((128, 968), dtype=bf)
    wblob[:, _WB_W1] = w1p.astype(bf)
    wblob[:, _WB_W2] = W2.astype(bf)
    wblob[:, _WB_W4] = w4p.astype(bf)
    wblob[0, _WB_ONE] = np.ones(128, dtype=bf)
    wblob[0, _WB_B4] = b4.astype(bf)
    fblob = np.zeros((128, 6), f)
    fblob[:, _FB_B1] = b1.astype(f).reshape(128, 1)
    fblob[:, _FB_B2] = b2.astype(f).reshape(4, 128).T
    # class consts per one-hot row; -1 on pad rows zeroes them via is_equal
    cvec = np.full(128, -1.0, f)
    cvec[32:36] = np.arange(4)
    cvec[36:43] = np.arange(1, 8)
    cvec[43:50] = np.arange(1, 8)
    cvec[50:57] = np.arange(1, 8)
    fblob[:, _FB_CV] = cvec.reshape(128, 1)
    shared = dict(wblob=wblob, fblob=fblob)
    in_maps = []
    for c in range(N_CORES):
        m = dict(shared)
        m["xrep"] = np.ascontiguousarray(xrep[c * B_CORE:(c + 1) * B_CORE])
        in_maps.append(m)
    return in_maps


def _get_nc(mode="bf16"):
    if mode not in _BUILT:
        _BUILT[mode] = _build(mode)
    return _BUILT[mode]


def run(x, W1, b1, W2, b2, W4, b4, mode="bf16", **kw):
    nc = _get_nc(mode)
    in_maps = _prep_inputs(x, W1, b1, W2, b2, W4, b4)
    res = run_bass_kernel_spmd(nc, in_maps, core_ids=list(range(N_CORES)), **kw)
    out = np.concatenate([r["out"] for r in res.results], axis=0)
    return out, res


def kernel(x, W1, b1, W2, b2, W4, b4):
    out, _ = run(x, W1, b1, W2, b2, W4, b4)
    return out
